# revision 4
# baseline (speedup 1.0000x reference)
"""Trainium2 Bass kernel for nn_AttentionLayer (B=64, S=512, F=256), 8 cores.

Reference computation (per batch b):
    scores = x1 @ Wq + x2 @ Wk          # [S, S]
    a = softmax(tanh(scores), axis=-1)   # softmax over u
    a2 = a @ Wv                          # [S, S]
    out = a2 * x1                        # elementwise
    out = out * rsqrt(max(sum_s out^2, eps))   # l2-normalize over axis s

Strategy: pure data parallelism -- 8 batches per core, weights replicated.
Everything is computed in a TRANSPOSED layout ([t-or-u partitions, s free]).

Design notes (informed by HW traces; v2 of the schedule):
  * x1 is DMA'd ONCE as float32r: stage A streams it as the moving matmul
    operand (1 cycle/row -- same PE rate as bf16), and the epilogue
    bitcasts the same SBUF bytes to f32.
  * All weights + x2 are bf16; batch 0 runs stage A from an all-bf16
    operand set (x1b0/wqb) streamed as k-strips so the PE starts after
    only ~0.25MB of DMA, k-outer so each strip is consumed on arrival.
  * Stage C consumes the UNNORMALIZED expz; 1/denominator folds into the
    epilogue, so no PE work waits on the rowsum->recip->broadcast chain.
  * The softmax rowsum uses a ones-BLOCK stationary so the matmul writes
    the denominator replicated across all 128 partitions.
  * Finalizes are flushed in groups -- (b0,b1), (b2,b3), (b4,b5,b6),
    then b7 at the drain -- with each group's ACT sqrts adjacent: one
    sqrt-table epoch per group.  Square lives in BOTH activation-table
    sets, so after the (b4,b5,b6) sqrt swap the drain's squares+sqrt
    need no further table load (6 table loads total vs 12).
  * The (b4,b5,b6) flush normalizes entirely on GpSimd so the DVE is
    free for b6's+b7's q/w chains in the drain window; mid-stream
    flushes split norms GpSimd/DVE.
  * Drain: no dummy matmuls -- C(b6) covers exp(b7) latency; b7's
    squares all go to ACT (idle after exp1), norms split GpSimd/DVE,
    output leaves in quarter-DMAs on alternating queues as tiles
    normalize.
  * All DRAM tensors partition-major; output bf16, upcast on host.
"""

import sys

sys.path.insert(0, "/opt/trn_rl_repo")

import numpy as np
import ml_dtypes

import concourse.bass as bass
import concourse.tile as tile
from concourse import bacc, mybir
from concourse.bass_utils import run_bass_kernel_spmd

B, S, F = 64, 512, 256
N_CORES = 8
BPC = B // N_CORES  # batches per core
P = 128
KT1 = S // P  # 4 k-tiles over t (x1/Wq contraction)
KT2 = F // P  # 2 k-tiles over f (x2/Wk contraction)
NT = S // P  # 4 m-tiles over u (stage A) / t (stage C)
EPS = 1e-12

F32 = mybir.dt.float32
F32R = mybir.dt.float32r
BF16 = mybir.dt.bfloat16
AF = mybir.ActivationFunctionType
ALU = mybir.AluOpType

BFNP = ml_dtypes.bfloat16

last_results = None  # test harness introspection


def build_nc(reps=1, bpc=BPC):
    nc = bacc.Bacc(
        "TRN2", target_bir_lowering=False, debug=False, num_devices=N_CORES
    )
    # Partition-major packed tensors: [.., P, ktiles, S].
    x1t = nc.declare_dram_parameter("x1t", [bpc, P, KT1, S], F32R, isOutput=False)
    x2t = nc.declare_dram_parameter("x2t", [bpc, P, KT2, S], BF16, isOutput=False)
    wq = nc.declare_dram_parameter("wq", [P, KT1, S], F32R, isOutput=False)
    wqb = nc.declare_dram_parameter("wqb", [P, KT1, S], BF16, isOutput=False)
    x1b0 = nc.declare_dram_parameter("x1b0", [P, KT1, S], BF16, isOutput=False)
    wk = nc.declare_dram_parameter("wk", [P, KT2, S], BF16, isOutput=False)
    wv = nc.declare_dram_parameter("wv", [P, NT, S], BF16, isOutput=False)
    out = nc.declare_dram_parameter("out", [bpc, P, NT, S], BF16, isOutput=True)

    batches = [bb for _ in range(reps) for bb in range(bpc)]
    nb = len(batches)

    with tile.TileContext(nc) as tc:
        with (
            tc.tile_pool(name="singles", bufs=1) as singles,
            tc.tile_pool(name="xin", bufs=1) as xin,
            tc.tile_pool(name="work", bufs=2) as work,
            tc.tile_pool(name="small", bufs=2) as small,
            tc.tile_pool(name="outp", bufs=2) as outp,
            tc.tile_pool(name="psA", bufs=2, space="PSUM") as psA,
            tc.tile_pool(name="psY", bufs=3, space="PSUM") as psY,
            tc.tile_pool(name="psR", bufs=1, space="PSUM") as psR,
        ):
            b0 = batches[0]
            # constants first (vector queue) so nothing blocks on them
            ones_blk = singles.tile([P, P], BF16)
            nc.vector.memset(ones_blk, 1.0)
            eps_t = singles.tile([P, 1], F32)
            nc.vector.memset(eps_t, EPS)

            # Startup: batch-0 operands land as k-strips, round-robin over
            # four queues, in the exact order the k-outer stage A consumes
            # them -- the PE starts after ~0.25MB instead of 1.5MB.
            wqb_t = singles.tile([P, KT1, S], BF16, tag="wqb")
            x1b0_t = xin.tile([P, KT1, S], BF16, tag="x1b0", bufs=1)
            wk_t = singles.tile([P, KT2, S], BF16, tag="wk")
            x2_first = xin.tile([P, KT2, S], BF16, tag="x2", bufs=3)
            strip_qs = [nc.sync, nc.scalar, nc.gpsimd]
            qi = 0
            for kt in range(KT1):
                strip_qs[qi % 3].dma_start(
                    out=wqb_t[:, kt : kt + 1, :], in_=wqb.ap()[:, kt : kt + 1, :]
                )
                strip_qs[(qi + 1) % 3].dma_start(
                    out=x1b0_t[:, kt : kt + 1, :],
                    in_=x1b0.ap()[:, kt : kt + 1, :],
                )
                qi += 2
            for kf in range(KT2):
                strip_qs[qi % 3].dma_start(
                    out=wk_t[:, kf : kf + 1, :], in_=wk.ap()[:, kf : kf + 1, :]
                )
                strip_qs[(qi + 1) % 3].dma_start(
                    out=x2_first[:, kf : kf + 1, :],
                    in_=x2t.ap()[b0, :, kf : kf + 1, :],
                )
                qi += 2
            # bulk tail: wv (stage C), wq f32r (batches 1+), x1 f32r (b0
            # epilogue) stream in behind the strips.
            wv_t = singles.tile([P, NT, S], BF16, tag="wv")
            nc.gpsimd.dma_start(out=wv_t, in_=wv.ap())
            wq_t = singles.tile([P, KT1, S], F32R, tag="wq")
            nc.scalar.dma_start(out=wq_t[:, 0:2, :], in_=wq.ap()[:, 0:2, :])
            nc.scalar.dma_start(out=wq_t[:, 2:4, :], in_=wq.ap()[:, 2:4, :])
            x1_first = xin.tile([P, KT1, S], F32R, tag="x1", bufs=3)
            nc.sync.dma_start(out=x1_first[:, 0:2, :], in_=x1t.ap()[b0, :, 0:2, :])
            nc.sync.dma_start(out=x1_first[:, 2:4, :], in_=x1t.ap()[b0, :, 2:4, :])

            def stage_a_b0():
                """k-outer stage A for batch 0: each (wqb strip, x1b0 strip)
                pair is consumed right as it lands; both u-pair PSUM tiles
                accumulate simultaneously."""
                sc01 = psA.tile([P, 2, S], F32, tag="scores")
                sc23 = psA.tile([P, 2, S], F32, tag="scores")
                scs = (sc01, sc23)
                for kt in range(KT1):
                    for half in range(2):
                        for j in range(2):
                            ut = half * 2 + j
                            us = slice(ut * P, (ut + 1) * P)
                            nc.tensor.matmul(
                                scs[half][:, j, :],
                                wqb_t[:, kt, us],
                                x1b0_t[:, kt, :],
                                start=(kt == 0),
                                stop=False,
                            )
                for kf in range(KT2):
                    for half in range(2):
                        for j in range(2):
                            ut = half * 2 + j
                            us = slice(ut * P, (ut + 1) * P)
                            nc.tensor.matmul(
                                scs[half][:, j, :],
                                wk_t[:, kf, us],
                                x2_first[:, kf, :],
                                start=False,
                                stop=(kf == KT2 - 1),
                            )
                expz = work.tile([P, NT, S], BF16, tag="expz", bufs=3)
                for half in range(2):
                    tanh_t = work.tile([P, 2, S], F32, tag="tanh")
                    nc.scalar.activation(out=tanh_t, in_=scs[half], func=AF.Tanh)
                    nc.scalar.activation(
                        out=expz[:, half * 2 : half * 2 + 2, :],
                        in_=tanh_t,
                        func=AF.Exp,
                    )
                return expz

            def stage_a(b, x1_sb, x2_sb, mid_cb=None):
                """scores matmuls in u-tile pairs sharing one 2-bank PSUM
                tile, tanh+exp over pairs.  mid_cb (if set) is emitted
                between the two pair-halves so the previous batch's rowsum
                overlaps this batch's remaining matmuls."""
                expz = work.tile([P, NT, S], BF16, tag="expz", bufs=3)
                for half in range(NT // 2):
                    sc = psA.tile([P, 2, S], F32, tag="scores")
                    for j in range(2):
                        ut = half * 2 + j
                        us = slice(ut * P, (ut + 1) * P)
                        prods = [
                            (wq_t[:, kt, us], x1_sb[:, kt, :]) for kt in range(KT1)
                        ] + [(wk_t[:, kt, us], x2_sb[:, kt, :]) for kt in range(KT2)]
                        for pi, (l_ap, r_ap) in enumerate(prods):
                            nc.tensor.matmul(
                                sc[:, j, :],
                                l_ap,
                                r_ap,
                                start=(pi == 0),
                                stop=(pi == len(prods) - 1),
                            )
                    tanh_t = work.tile([P, 2, S], F32, tag="tanh")
                    nc.scalar.activation(out=tanh_t, in_=sc, func=AF.Tanh)
                    nc.scalar.activation(
                        out=expz[:, half * 2 : half * 2 + 2, :],
                        in_=tanh_t,
                        func=AF.Exp,
                    )
                    if half == 0 and mid_cb is not None:
                        mid_cb()
                return expz

            def stage_b(b, expz):
                """softmax denominator: the ones-BLOCK rowsum matmul writes
                the denominator replicated across all 128 partitions, so the
                reciprocal lands directly in broadcast form."""
                rs = psR.tile([P, S], F32, tag="rowsum")
                for ut in range(NT):
                    nc.tensor.matmul(
                        rs,
                        ones_blk,
                        expz[:, ut, :],
                        start=(ut == 0),
                        stop=(ut == NT - 1),
                    )
                rbc = small.tile([P, S], F32, tag="rbc")
                nc.vector.reciprocal_approx_fast(out=rbc, in_=rs)
                return rbc

            def stage_c(b, x1_sb, expz, rbc, sq_act_all=False):
                """Y matmuls on raw expz; epilogue q=y*x1 -> w=q*rbc (f32 on
                DVE); sum-of-squares split between ACT Square+accum and DVE
                stt (or all-ACT in the drain, when ACT is idle post-exp and
                the DVE is the critical chain)."""
                w_sb = outp.tile([P, NT, S], F32, tag="w", bufs=4)
                sumsq = small.tile([P, NT], F32, tag="sumsq", bufs=5)
                for tt in range(NT):
                    y = psY.tile([P, S], F32, tag="y")
                    for ut in range(NT):
                        nc.tensor.matmul(
                            y,
                            wv_t[:, ut, tt * P : (tt + 1) * P],
                            expz[:, ut, :],
                            start=(ut == 0),
                            stop=(ut == NT - 1),
                        )
                    q_t = small.tile([P, S], F32, tag="q")
                    w_t = w_sb[:, tt, :]
                    nc.vector.tensor_tensor(
                        out=q_t, in0=y, in1=x1_sb[:, tt, :].bitcast(F32), op=ALU.mult
                    )
                    nc.vector.tensor_tensor(out=w_t, in0=q_t, in1=rbc, op=ALU.mult)
                    if tt >= 2 and not sq_act_all:
                        scr = small.tile([P, S], F32, tag="scr")
                        nc.vector.scalar_tensor_tensor(
                            out=scr,
                            in0=w_t,
                            scalar=1.0,
                            in1=w_t,
                            op0=ALU.mult,
                            op1=ALU.mult,
                            accum_out=sumsq[:, tt : tt + 1],
                        )
                    else:
                        scr = small.tile([P, S], BF16, tag="scrb")
                        nc.scalar.activation(
                            out=scr,
                            in_=w_t,
                            func=AF.Square,
                            accum_out=sumsq[:, tt : tt + 1],
                        )
                return w_sb, sumsq

            def stage_fin_group(fins, gpsimd_only=False):
                """Finalize a group of batches.  All ACT sqrts emitted
                adjacently = ONE sqrt-table epoch for the group.  Norms
                split GpSimd/DVE mid-stream; all-GpSimd for the pre-drain
                flush so the DVE stays free for the drain chain."""
                rsqs = []
                for (b, w_sb, sumsq) in fins:
                    rsq = small.tile([P, NT], F32, tag="rsq", bufs=5)
                    nc.scalar.activation(
                        out=rsq, in_=sumsq, func=AF.Sqrt, bias=eps_t
                    )
                    rsqs.append(rsq)
                for (b, w_sb, sumsq), rsq in zip(fins, rsqs):
                    ob = outp.tile([P, NT, S], BF16, tag="ob", bufs=4)
                    if gpsimd_only:
                        for tt in range(NT):
                            nc.gpsimd.normalize_recip(
                                out_ap=ob[:, tt, :],
                                in_ap=w_sb[:, tt, :],
                                denom_ap=rsq[:, tt : tt + 1],
                            )
                    else:
                        vv = small.tile([P, NT], F32, tag="vv", bufs=4)
                        nc.vector.reciprocal_approx_fast(out=vv, in_=rsq)
                        for tt in range(NT):
                            if tt < 2:
                                nc.gpsimd.normalize_recip(
                                    out_ap=ob[:, tt, :],
                                    in_ap=w_sb[:, tt, :],
                                    denom_ap=rsq[:, tt : tt + 1],
                                )
                            else:
                                nc.vector.tensor_scalar_mul(
                                    ob[:, tt, :],
                                    w_sb[:, tt, :],
                                    vv[:, tt : tt + 1],
                                )
                    nc.sync.dma_start(out=out.ap()[b], in_=ob)

            def stage_fin_last(b, w_sb, sumsq):
                """Drain finalize: norms split GpSimd/DVE, quarter-DMAs on
                alternating queues as tiles complete.  Uses the sqrt-set
                table epoch opened by the pre-drain flush (Square is in
                both sets, so no swap happens in between)."""
                rsq = small.tile([P, NT], F32, tag="rsq", bufs=5)
                nc.scalar.activation(out=rsq, in_=sumsq, func=AF.Sqrt, bias=eps_t)
                vv = small.tile([P, NT], F32, tag="vv", bufs=4)
                nc.vector.reciprocal_approx_fast(out=vv, in_=rsq)
                ob = outp.tile([P, NT, S], BF16, tag="ob", bufs=4)
                for tt in range(NT):
                    if tt % 2 == 0:
                        nc.gpsimd.normalize_recip(
                            out_ap=ob[:, tt, :],
                            in_ap=w_sb[:, tt, :],
                            denom_ap=rsq[:, tt : tt + 1],
                        )
                        nc.sync.dma_start(
                            out=out.ap()[b, :, tt : tt + 1, :],
                            in_=ob[:, tt : tt + 1, :],
                        )
                    else:
                        nc.vector.tensor_scalar_mul(
                            ob[:, tt, :], w_sb[:, tt, :], vv[:, tt : tt + 1]
                        )
                        nc.gpsimd.dma_start(
                            out=out.ap()[b, :, tt : tt + 1, :],
                            in_=ob[:, tt : tt + 1, :],
                        )

            def dma_x(b):
                t1 = xin.tile([P, KT1, S], F32R, tag="x1", bufs=3)
                nc.sync.dma_start(out=t1[:, 0:2, :], in_=x1t.ap()[b, :, 0:2, :])
                nc.sync.dma_start(out=t1[:, 2:4, :], in_=x1t.ap()[b, :, 2:4, :])
                t2 = xin.tile([P, KT2, S], BF16, tag="x2", bufs=3)
                nc.gpsimd.dma_start(out=t2, in_=x2t.ap()[b])
                return t1, t2

            # flush points: after stage_c of batch index i-1 at iteration i
            flush_at = {2: 2, 4: 2, nb - 1: 3} if nb >= 5 else {nb - 1: nb - 1}

            pending = None  # (b, x1_sb, expz) awaiting stages B+C
            fins = []  # (b, w_sb, sumsq) awaiting finalize
            x1_cur, x2_cur = x1_first, x2_first
            for i, b in enumerate(batches):
                if i + 1 < len(batches):
                    nxt = dma_x(batches[i + 1])
                else:
                    nxt = (None, None)
                prev = pending
                hold = {}

                def mid_cb():
                    hold["rbc"] = stage_b(prev[0], prev[2])

                if i == 0:
                    expz = stage_a_b0()
                else:
                    expz = stage_a(b, x1_cur, x2_cur, mid_cb)
                if prev is not None:
                    fins.append(
                        (prev[0],)
                        + stage_c(
                            prev[0], prev[1], prev[2], hold["rbc"],
                        )
                    )
                    if i in flush_at and len(fins) >= flush_at[i]:
                        stage_fin_group(fins, gpsimd_only=(i == nb - 1))
                        fins = []
                pending = (b, x1_cur, expz)
                x1_cur, x2_cur = nxt
            # drain: C(b6) (emitted above) covers exp(b7) latency -- no
            # dummy matmuls needed.  b7's squares all ride ACT (idle after
            # exp1); its norms+DMAs leave in quarters.
            rbc_last = stage_b(pending[0], pending[2])
            for f in fins:
                stage_fin_group([f], gpsimd_only=True)
            last_c = stage_c(
                pending[0], pending[1], pending[2], rbc_last, sq_act_all=True
            )
            stage_fin_last(pending[0], *last_c)

    nc.compile()
    return nc


def _pack_pmajor(a, nchunks):
    """[.., nchunks*P, S] -> [.., P, nchunks, S] partition-major contiguous."""
    lead = a.shape[:-2]
    a = a.reshape(lead + (nchunks, P, S))
    perm = tuple(range(len(lead))) + (len(lead) + 1, len(lead), len(lead) + 2)
    return np.ascontiguousarray(a.transpose(perm))


_nc_cache = None


def kernel(x1, x2, W_query, W_key, W_value, _trace=False):
    global _nc_cache, last_results
    x1t = _pack_pmajor(
        np.asarray(x1, dtype=np.float32).transpose(0, 2, 1), KT1
    )  # [B, P, KT1, S]
    x2t = _pack_pmajor(
        np.asarray(x2, dtype=np.float32).transpose(0, 2, 1).astype(BFNP), KT2
    )
    wq = _pack_pmajor(np.asarray(W_query, dtype=np.float32), KT1)
    wqb = _pack_pmajor(np.asarray(W_query, dtype=np.float32).astype(BFNP), KT1)
    x1b_all = _pack_pmajor(
        np.asarray(x1, dtype=np.float32).transpose(0, 2, 1).astype(BFNP), KT1
    )
    wk = _pack_pmajor(np.asarray(W_key, dtype=np.float32).astype(BFNP), KT2)
    wv = _pack_pmajor(np.asarray(W_value, dtype=np.float32).astype(BFNP), NT)

    if _nc_cache is None:
        _nc_cache = build_nc()
    nc = _nc_cache

    in_maps = []
    for c in range(N_CORES):
        sl = slice(c * BPC, (c + 1) * BPC)
        in_maps.append(
            {
                "x1t": x1t[sl],
                "x2t": x2t[sl],
                "wq": wq,
                "wqb": wqb,
                "x1b0": x1b_all[c * BPC],
                "wk": wk,
                "wv": wv,
            }
        )
    res = run_bass_kernel_spmd(
        nc, in_maps, core_ids=list(range(N_CORES)), trace=_trace
    )
    last_results = res
    # out: [bpc, P, NT, S] bf16 -> outT [B, S, S] -> untranspose
    outs = [np.asarray(res.results[c]["out"]) for c in range(N_CORES)]
    outT = np.concatenate(outs, axis=0).astype(np.float32)
    outT = outT.transpose(0, 2, 1, 3).reshape(B, S, S)
    return np.ascontiguousarray(outT.transpose(0, 2, 1))
